# revision 1
# baseline (speedup 1.0000x reference)
"""MHA (projections + masked softmax attention) on 8 NeuronCores.

Data-parallel over batch (B=8 -> 1 batch element per core, no collectives).
bf16 matmul operands (fp32 PSUM accumulation + fp32 softmax normalization).

Per core, transposed layout:
  QT = Wq^T @ x_q^T   [D, Sq]   (lhsT = Wq natural, rhs = x_q^T from host)
  KT = Wk^T @ x_k^T   [D, Sk]
  V  = x_v  @ Wv      [Sk, D]   (lhsT = x_v^T chunk, rhs = Wv natural)

Attention per head h in "scores transposed" layout S^T[k, q]:
  S^T = KT_h_chunk.T @ QT_h                (k on partitions, q free, N=1024)
  masked scores: copy_predicated(-480) then exp(0.125*s) => exp(-60) ~= 0
  O^T[d,q] & Z[q] in ONE accumulating matmul: lhsT = [V_h | ones] (65 cols)
  final: O = transpose(O^T) * (1/Z) per 128-query block, DMA per head.

Host: transposes, sort queries by valid_len (column-suffix skipping of
fully-masked key chunks + narrow predication ranges), uint8 mask, exact
fixup of valid_len==0 rows (reference -> uniform softmax -> mean(value)@Wv).
"""

import os
import sys

if "/opt/trn_rl_repo" not in sys.path:
    sys.path.insert(0, "/opt/trn_rl_repo")

import numpy as np

ABLATE = set(os.environ.get("ABLATE", "").split(","))

B, S, D, H = 8, 1024, 1024, 16
DH = D // H  # 64
P = 128
KC = S // P  # 8 key chunks
DC = D // P  # 8 hidden chunks
N_CORES = 8
NEG = -480.0  # exp(0.125 * -480) = exp(-60) ~= 8.8e-27


def _build_nc(col_start, pred_end, reps=1):
    """col_start[kc]: first sorted-q column (mult of 128, 0..1024) needing
    key-chunk kc (1024 = chunk skipped). pred_end[kc]: end (exclusive, mult
    of 128) of the mask-predication range. Unions over cores. col_start[0]
    must be 0."""
    from contextlib import ExitStack

    import concourse.mybir as mybir
    import concourse.tile as tile
    from concourse import bacc
    from concourse.masks import make_identity

    fp32 = mybir.dt.float32
    bf16 = mybir.dt.bfloat16
    u8 = mybir.dt.uint8
    AF = mybir.ActivationFunctionType

    nc = bacc.Bacc(
        "TRN2",
        target_bir_lowering=False,
        debug=False,
        enable_asserts=False,
        num_devices=N_CORES,
    )

    xqT = nc.dram_tensor("xqT", (D, S), bf16, kind="ExternalInput").ap()
    xkT = nc.dram_tensor("xkT", (D, S), bf16, kind="ExternalInput").ap()
    xvT = nc.dram_tensor("xvT", (D, S), bf16, kind="ExternalInput").ap()
    wq = nc.dram_tensor("wq", (D, D), bf16, kind="ExternalInput").ap()
    wk = nc.dram_tensor("wk", (D, D), bf16, kind="ExternalInput").ap()
    wv = nc.dram_tensor("wv", (D, D), bf16, kind="ExternalInput").ap()
    maskT = nc.dram_tensor("maskT", (S, S), u8, kind="ExternalInput").ap()
    out = nc.dram_tensor("out", (S, D), fp32, kind="ExternalOutput").ap()

    with ExitStack() as ctx:
        tc = ctx.enter_context(tile.TileContext(nc))
        const = ctx.enter_context(tc.tile_pool(name="const", bufs=1))
        persist = ctx.enter_context(tc.tile_pool(name="persist", bufs=1))
        wpool = ctx.enter_context(tc.tile_pool(name="wpool", bufs=1))
        ppool = ctx.enter_context(tc.tile_pool(name="ppool", bufs=1, space="PSUM"))
        epool = ctx.enter_context(tc.tile_pool(name="epool", bufs=4))
        mpool = ctx.enter_context(tc.tile_pool(name="mpool", bufs=3))

        NB = 512  # max psum-bank columns (fp32) per matmul

        def mm(out_ap, lhsT, rhs, base, start, stop):
            # split a wide matmul into <=512-col pieces so each PE write
            # stays inside one PSUM bank. base = column offset of out_ap[0]
            # within its tile (bank alignment reference).
            w = rhs.shape[-1]
            off = 0
            while off < w:
                step = min(NB - ((base + off) % NB), w - off)
                nc.tensor.matmul(
                    out_ap[:, off : off + step],
                    lhsT,
                    rhs[:, off : off + step],
                    start=start,
                    stop=stop,
                )
                off += step

        ident = const.tile([P, P], fp32)
        make_identity(nc, ident[:])
        negt = const.tile([P, S], fp32)
        nc.gpsimd.memset(negt[:], NEG)

        rep_cm = tc.For_i(0, reps, 1) if reps > 1 else None
        if rep_cm is not None:
            ctx.enter_context(rep_cm)

        qt_sb = [persist.tile([P, S], bf16, tag=f"qt{i}", name=f"qt{i}") for i in range(DC)]
        kt_sb = [persist.tile([P, S], bf16, tag=f"kt{i}", name=f"kt{i}") for i in range(DC)]
        va_sb = [persist.tile([P, H * (DH + 1)], bf16, tag=f"va{i}", name=f"va{i}") for i in range(KC)]
        mk_sb = [persist.tile([P, S], u8, tag=f"mk{i}", name=f"mk{i}") for i in range(KC)]
        for kc in range(KC):
            nc.sync.dma_start(mk_sb[kc][:], maskT[kc * P : (kc + 1) * P, :])
            va3 = va_sb[kc].rearrange("p (h d) -> p h d", d=DH + 1)
            nc.vector.memset(va3[:, :, DH], 1.0)

        # ---- projections. x and W fully resident (bf16, 2MB each) ----
        def load_x(x_dram, pfx):
            xf = [wpool.tile([P, S], bf16, tag=f"x{pfx}{i}", name=f"x{pfx}{i}") for i in range(DC)]
            for dc in range(DC):
                nc.sync.dma_start(xf[dc][:], x_dram[dc * P : (dc + 1) * P, :])
            return xf

        def load_w(w_dram, pfx):
            w_sb = [wpool.tile([P, D], bf16, tag=f"w{pfx}{i}", name=f"w{pfx}{i}") for i in range(DC)]
            for dc in range(DC):
                nc.sync.dma_start(w_sb[dc][:], w_dram[dc * P : (dc + 1) * P, :])
            return w_sb

        def project_t(w_sb, xf, dst_sb, evac_engine):
            # out[d, q] = W^T @ xT ; per out-chunk: acc[128, 1024] over dc
            for oc in range(DC):
                acc = ppool.tile([P, S], fp32, tag=f"pj{oc % 2}", name="acc")
                for dc in range(DC):
                    if "nomm" in ABLATE:
                        break
                    mm(acc[:], w_sb[dc][:, oc * P : (oc + 1) * P], xf[dc][:],
                       0, dc == 0, dc == DC - 1)
                if "nomm" not in ABLATE and "noevac" not in ABLATE:
                    if evac_engine == "act":
                        nc.scalar.copy(dst_sb[oc][:], acc[:])
                    else:
                        nc.vector.tensor_copy(dst_sb[oc][:], acc[:])

        if "noproj" not in ABLATE:
            xf = load_x(xqT, "q")
            w_sb = load_w(wq, "q")
            project_t(w_sb, xf, qt_sb, "act")
            xf = load_x(xkT, "k")
            w_sb = load_w(wk, "k")
            project_t(w_sb, xf, kt_sb, "dve")
            # V: out[k, d] tiles; lhsT = xvT chunk [hid, k], rhs = Wv [hid, d]
            xf = load_x(xvT, "v")
            w_sb = load_w(wv, "v")
            for kc in range(KC):
                acc = ppool.tile([P, S], fp32, tag=f"pj{kc % 2}", name="vacc")
                for dc in range(DC):
                    if "nomm" in ABLATE:
                        break
                    mm(acc[:], xf[dc][:, kc * P : (kc + 1) * P], w_sb[dc][:],
                       0, dc == 0, dc == DC - 1)
                if "nomm" not in ABLATE:
                    dst = va_sb[kc].rearrange("p (h d) -> p h d", d=DH + 1)[:, :, 0:DH]
                    nc.scalar.copy(dst, acc[:].rearrange("p (h d) -> p h d", d=DH))

        # ---- attention ----
        kcs = [kc for kc in range(KC) if col_start[kc] < S]
        for h in (range(H) if "noattn" not in ABLATE else []):
            oc, ro = h // 2, (h % 2) * DH
            att = ppool.tile([DH + 1, S], fp32, tag=f"pj{h % 2}", name="att")
            for i, kc in enumerate(kcs):
                c0 = col_start[kc]
                cv = pred_end[kc]
                sc = ppool.tile([P, S], fp32, tag=f"sc{i % 2}", name="sc")
                mm(sc[:, c0:], kt_sb[oc][ro : ro + DH, kc * P : (kc + 1) * P],
                   qt_sb[oc][ro : ro + DH, c0:], c0, True, True)
                e = epool.tile([P, S], bf16, tag="e")
                if cv > c0 and "nopred" not in ABLATE:
                    nc.vector.copy_predicated(
                        sc[:, c0:cv], mk_sb[kc][:, c0:cv], negt[:, : cv - c0]
                    )
                if "noexp" not in ABLATE:
                    if cv > c0:
                        nc.scalar.activation(
                            e[:, c0:cv], sc[:, c0:cv], AF.Exp, scale=0.125
                        )
                    if cv < S:
                        nc.scalar.activation(
                            e[:, cv:], sc[:, cv:], AF.Exp, scale=0.125
                        )
                else:
                    nc.scalar.copy(e[:, c0:], sc[:, c0:])
                mm(att[:, c0:], va_sb[kc][:, h * (DH + 1) : (h + 1) * (DH + 1)],
                   e[:, c0:], c0, i == 0, i == len(kcs) - 1)
            if "notr" in ABLATE:
                continue
            # att rows 0:64 = O^T unnormalized, row 64 = Z
            asb = mpool.tile([DH + 1, S], fp32, tag="asb")
            nc.vector.tensor_copy(asb[:], att[:])
            trs = [
                ppool.tile([P, 4 * (DH + 1)], fp32, tag=f"pj{h % 2}", name="tra"),
            ]
            trs.append(
                ppool.tile([P, 4 * (DH + 1)], fp32, tag=f"sc{h % 2}", name="trb")
            )
            for s in range(KC):
                nc.tensor.transpose(
                    trs[s // 4][:, (s % 4) * (DH + 1) : (s % 4 + 1) * (DH + 1)],
                    asb[:, s * P : (s + 1) * P],
                    ident[: DH + 1, : DH + 1],
                )
            rz = mpool.tile([P, KC], fp32, tag="rz")
            tr3a = trs[0].rearrange("p (s d) -> p s d", d=DH + 1)
            tr3b = trs[1].rearrange("p (s d) -> p s d", d=DH + 1)
            nc.vector.reciprocal(rz[:, 0:4], tr3a[:, :, DH])
            nc.vector.reciprocal(rz[:, 4:8], tr3b[:, :, DH])
            for s in range(KC):
                t3 = tr3a if s < 4 else tr3b
                ot = mpool.tile([P, DH], fp32, tag="ot")
                nc.vector.tensor_scalar_mul(ot[:], t3[:, s % 4, 0:DH], rz[:, s : s + 1])
                nc.sync.dma_start(
                    out[s * P : (s + 1) * P, h * DH : (h + 1) * DH], ot[:]
                )

    nc.compile()
    return nc


_NC_CACHE = {}
_LAST_IN_MAPS = None


def _get_nc(col_start, pred_end):
    key = (tuple(col_start), tuple(pred_end))
    if key not in _NC_CACHE:
        _NC_CACHE[key] = _build_nc(list(col_start), list(pred_end))
    return _NC_CACHE[key]


def _prep(query, key, value, valid_len, Wq, Wk, Wv):
    import ml_dtypes

    bf = ml_dtypes.bfloat16
    kidx = np.arange(S, dtype=np.int32)
    orders = []
    in_maps = []
    col_start = [S] * KC
    pred_end = [0] * KC
    wqb, wkb, wvb = Wq.astype(bf), Wk.astype(bf), Wv.astype(bf)
    for b in range(B):
        vl = valid_len[b]
        vl2 = np.where(vl == 0, 1, vl).astype(np.int32)
        order = np.argsort(vl2, kind="stable")
        orders.append(order)
        vs = vl2[order]
        for kc in range(KC):
            need = vs > (kc * P)
            c0 = S if not need.any() else (int(np.argmax(need)) // P) * P
            col_start[kc] = min(col_start[kc], c0)
            full = vs >= ((kc + 1) * P)
            cv = S if not full.any() else int(np.argmax(full))
            pred_end[kc] = max(pred_end[kc], min(S, -(-cv // 32) * 32))
        in_maps.append(
            {
                "xqT": np.ascontiguousarray(query[b][order].T.astype(bf)),
                "xkT": np.ascontiguousarray(key[b].T.astype(bf)),
                "xvT": np.ascontiguousarray(value[b].T.astype(bf)),
                "wq": wqb,
                "wk": wkb,
                "wv": wvb,
                "maskT": (kidx[:, None] >= vs[None, :]).astype(np.uint8),
            }
        )
    return in_maps, orders, col_start, pred_end


def kernel(query, key, value, valid_len, Wq, Wk, Wv):
    from concourse import bass_utils

    query = np.asarray(query, dtype=np.float32)
    key = np.asarray(key, dtype=np.float32)
    value = np.asarray(value, dtype=np.float32)
    valid_len = np.asarray(valid_len, dtype=np.int32)
    Wq = np.asarray(Wq, dtype=np.float32)
    Wk = np.asarray(Wk, dtype=np.float32)
    Wv = np.asarray(Wv, dtype=np.float32)

    in_maps, orders, col_start, pred_end = _prep(
        query, key, value, valid_len, Wq, Wk, Wv
    )
    nc = _get_nc(col_start, pred_end)
    global _LAST_IN_MAPS
    _LAST_IN_MAPS = in_maps
    res = bass_utils.run_bass_kernel_spmd(nc, in_maps, core_ids=list(range(N_CORES)))

    outs = np.empty((B, S, D), dtype=np.float32)
    for b in range(B):
        o_sorted = res.results[b]["out"]
        inv = np.empty(S, dtype=np.int64)
        inv[orders[b]] = np.arange(S)
        outs[b] = o_sorted[inv]
        zrows = np.where(valid_len[b] == 0)[0]
        if len(zrows):
            outs[b][zrows] = value[b].mean(axis=0) @ Wv
    return outs



# revision 2
# speedup vs baseline: 1.4182x; 1.4182x over previous
"""MHA (projections + masked softmax attention) on 8 NeuronCores.

Data-parallel over batch (B=8 -> 1 batch element per core, no collectives).
bf16 matmul operands (fp32 PSUM accumulation + fp32 softmax normalization).

Per core, transposed layout:
  QT = Wq^T @ x_q^T   [D, Sq]   (lhsT = Wq natural, rhs = x_q^T from host)
  KT = Wk^T @ x_k^T   [D, Sk]
  V  = x_v  @ Wv      [Sk, D]   (lhsT = x_v^T chunk, rhs = Wv natural)

Attention per head h in "scores transposed" layout S^T[k, q]:
  S^T = KT_h_chunk.T @ QT_h                (k on partitions, q free, N=1024)
  masked scores: copy_predicated(-480) then exp(0.125*s) => exp(-60) ~= 0
  O^T[d,q] & Z[q] in ONE accumulating matmul: lhsT = [V_h | ones] (65 cols)
  raw [O^T ; Z] rows DMA'd to HBM; the host divides O^T by Z, transposes,
  and un-sorts the queries (cheaper than on-chip transpose+normalize, and
  it keeps the PE stream dense so the HAM clock stays warm).

Host: transposes, sort queries by valid_len (column-suffix skipping of
fully-masked key chunks + narrow predication ranges), uint8 mask, exact
fixup of valid_len==0 rows (reference -> uniform softmax -> mean(value)@Wv).
"""

import os
import sys

if "/opt/trn_rl_repo" not in sys.path:
    sys.path.insert(0, "/opt/trn_rl_repo")

import numpy as np

ABLATE = set(os.environ.get("ABLATE", "").split(","))

B, S, D, H = 8, 1024, 1024, 16
DH = D // H  # 64
P = 128
KC = S // P  # 8 key chunks
DC = D // P  # 8 hidden chunks
N_CORES = 8
NEG = -480.0  # exp(0.125 * -480) = exp(-60) ~= 8.8e-27


def _build_nc(col_start, pred_end, reps=1):
    """col_start[kc]: first sorted-q column (mult of 128, 0..1024) needing
    key-chunk kc (1024 = chunk skipped). pred_end[kc]: end (exclusive, mult
    of 32) of the mask-predication range. Unions over cores. col_start[0]
    must be 0."""
    from contextlib import ExitStack

    import concourse.mybir as mybir
    import concourse.tile as tile
    from concourse import bacc

    fp32 = mybir.dt.float32
    bf16 = mybir.dt.bfloat16
    u8 = mybir.dt.uint8
    AF = mybir.ActivationFunctionType

    nc = bacc.Bacc(
        "TRN2",
        target_bir_lowering=False,
        debug=False,
        enable_asserts=False,
        num_devices=N_CORES,
    )

    xqT = nc.dram_tensor("xqT", (D, S), bf16, kind="ExternalInput").ap()
    xkT = nc.dram_tensor("xkT", (D, S), bf16, kind="ExternalInput").ap()
    xvT = nc.dram_tensor("xvT", (D, S), bf16, kind="ExternalInput").ap()
    wq = nc.dram_tensor("wq", (D, D), bf16, kind="ExternalInput").ap()
    wk = nc.dram_tensor("wk", (D, D), bf16, kind="ExternalInput").ap()
    wv = nc.dram_tensor("wv", (D, D), bf16, kind="ExternalInput").ap()
    maskT = nc.dram_tensor("maskT", (S, S), u8, kind="ExternalInput").ap()
    # raw per-head output: rows 0:64 = unnormalized O^T, row 64 = Z
    outT = nc.dram_tensor("outT", (H, DH + 1, S), fp32, kind="ExternalOutput").ap()

    with ExitStack() as ctx:
        tc = ctx.enter_context(tile.TileContext(nc))
        const = ctx.enter_context(tc.tile_pool(name="const", bufs=1))
        persist = ctx.enter_context(tc.tile_pool(name="persist", bufs=1))
        wpool = ctx.enter_context(tc.tile_pool(name="wpool", bufs=1))
        ppool = ctx.enter_context(tc.tile_pool(name="ppool", bufs=1, space="PSUM"))
        epool = ctx.enter_context(tc.tile_pool(name="epool", bufs=4))
        mpool = ctx.enter_context(tc.tile_pool(name="mpool", bufs=3))

        NB = 512  # max psum-bank columns (fp32) per matmul

        def mm(out_ap, lhsT, rhs, base, start, stop):
            # split a wide matmul into <=512-col pieces so each PE write
            # stays inside one PSUM bank. base = column offset of out_ap[0]
            # within its tile (bank alignment reference).
            w = rhs.shape[-1]
            off = 0
            while off < w:
                step = min(NB - ((base + off) % NB), w - off)
                nc.tensor.matmul(
                    out_ap[:, off : off + step],
                    lhsT,
                    rhs[:, off : off + step],
                    start=start,
                    stop=stop,
                )
                off += step

        negt = const.tile([P, S], fp32)
        nc.gpsimd.memset(negt[:], NEG)
        wrm = const.tile([P, NB], bf16)
        nc.vector.memset(wrm[:], 0.5)

        rep_cm = tc.For_i(0, reps, 1) if reps > 1 else None
        if rep_cm is not None:
            ctx.enter_context(rep_cm)

        # HAM warmup: ~10 back-to-back dummy matmuls keep the PE busy
        # through the initial input-DMA window so the clock gate opens
        # (K=8/8) before the real matmul stream starts.
        if "nowarm" not in ABLATE:
            wps = ppool.tile([P, NB], fp32, tag="sc0", name="warm")
            for _ in range(10):
                nc.tensor.matmul(wps[:], wrm[:, :P], wrm[:], start=True, stop=True)

        qt_sb = [persist.tile([P, S], bf16, tag=f"qt{i}", name=f"qt{i}") for i in range(DC)]
        kt_sb = [persist.tile([P, S], bf16, tag=f"kt{i}", name=f"kt{i}") for i in range(DC)]
        va_sb = [persist.tile([P, H * (DH + 1)], bf16, tag=f"va{i}", name=f"va{i}") for i in range(KC)]
        mk_sb = [persist.tile([P, S], u8, tag=f"mk{i}", name=f"mk{i}") for i in range(KC)]

        # ---- projections. x and W fully resident (bf16, 2MB each) ----
        def load_x(x_dram, pfx):
            xf = [wpool.tile([P, S], bf16, tag=f"x{pfx}{i}", name=f"x{pfx}{i}") for i in range(DC)]
            for dc in range(DC):
                nc.sync.dma_start(xf[dc][:], x_dram[dc * P : (dc + 1) * P, :])
            return xf

        def load_w(w_dram, pfx):
            w_sb = [wpool.tile([P, D], bf16, tag=f"w{pfx}{i}", name=f"w{pfx}{i}") for i in range(DC)]
            for dc in range(DC):
                nc.sync.dma_start(w_sb[dc][:], w_dram[dc * P : (dc + 1) * P, :])
            return w_sb

        def project_t(w_sb, xf, dst_sb, oc, evac_engine):
            # out[d, q] = W^T @ xT for one 128-row out-chunk: acc over dc
            acc = ppool.tile([P, S], fp32, tag=f"pj{oc % 2}", name="acc")
            for dc in range(DC):
                if "nomm" in ABLATE:
                    break
                mm(acc[:], w_sb[dc][:, oc * P : (oc + 1) * P], xf[dc][:],
                   0, dc == 0, dc == DC - 1)
            if "nomm" not in ABLATE and "noevac" not in ABLATE:
                if evac_engine == "act":
                    nc.scalar.copy(dst_sb[oc][:], acc[:])
                else:
                    nc.vector.tensor_copy(dst_sb[oc][:], acc[:])

        kcs = [kc for kc in range(KC) if col_start[kc] < S]

        def attend(h):
            oc, ro = h // 2, (h % 2) * DH
            att = ppool.tile([DH + 1, S], fp32, tag=f"pj{h % 2}", name="att")
            for i, kc in enumerate(kcs):
                c0 = col_start[kc]
                cv = pred_end[kc]
                sc = ppool.tile([P, S], fp32, tag=f"sc{i % 2}", name="sc")
                mm(sc[:, c0:], kt_sb[oc][ro : ro + DH, kc * P : (kc + 1) * P],
                   qt_sb[oc][ro : ro + DH, c0:], c0, True, True)
                e = epool.tile([P, S], bf16, tag="e")
                if cv > c0 and "nopred" not in ABLATE:
                    nc.vector.copy_predicated(
                        sc[:, c0:cv], mk_sb[kc][:, c0:cv], negt[:, : cv - c0]
                    )
                if "noexp" not in ABLATE:
                    nc.scalar.activation(e[:, c0:], sc[:, c0:], AF.Exp, scale=0.125)
                else:
                    nc.scalar.copy(e[:, c0:], sc[:, c0:])
                mm(att[:, c0:], va_sb[kc][:, h * (DH + 1) : (h + 1) * (DH + 1)],
                   e[:, c0:], c0, i == 0, i == len(kcs) - 1)
            # evacuate raw [O^T ; Z] and ship to HBM; host normalizes.
            asb = mpool.tile([DH + 1, S], fp32, tag="asb")
            if h % 2 == 0:
                nc.vector.tensor_copy(asb[:], att[:])
            else:
                nc.scalar.copy(asb[:], att[:])
            nc.sync.dma_start(outT[h], asb[:])

        if "noproj" not in ABLATE:
            xf = load_x(xkT, "k")
            w_sb = load_w(wk, "k")
            for oc in range(DC):
                project_t(w_sb, xf, kt_sb, oc, "dve")
            # V: out[k, d] tiles; lhsT = xvT chunk [hid, k], rhs = Wv natural
            xf = load_x(xvT, "v")
            w_sb = load_w(wv, "v")
            for kc in range(KC):
                va3 = va_sb[kc].rearrange("p (h d) -> p h d", d=DH + 1)
                nc.vector.memset(va3[:, :, DH], 1.0)
                acc = ppool.tile([P, S], fp32, tag=f"pj{kc % 2}", name="vacc")
                for dc in range(DC):
                    if "nomm" in ABLATE:
                        break
                    mm(acc[:], xf[dc][:, kc * P : (kc + 1) * P], w_sb[dc][:],
                       0, dc == 0, dc == DC - 1)
                if "nomm" not in ABLATE:
                    dst = va3[:, :, 0:DH]
                    nc.scalar.copy(dst, acc[:].rearrange("p (h d) -> p h d", d=DH))
            for kc in range(KC):
                nc.sync.dma_start(mk_sb[kc][:], maskT[kc * P : (kc + 1) * P, :])
            xf = load_x(xqT, "q")
            w_sb = load_w(wq, "q")
            for oc in range(DC):
                project_t(w_sb, xf, qt_sb, oc, "act")
                if "noattn" not in ABLATE:
                    attend(2 * oc)
                    attend(2 * oc + 1)

    nc.compile()
    return nc


_NC_CACHE = {}
_LAST_IN_MAPS = None


def _get_nc(col_start, pred_end):
    key = (tuple(col_start), tuple(pred_end))
    if key not in _NC_CACHE:
        _NC_CACHE[key] = _build_nc(list(col_start), list(pred_end))
    return _NC_CACHE[key]


def _prep(query, key, value, valid_len, Wq, Wk, Wv):
    import ml_dtypes

    bf = ml_dtypes.bfloat16
    kidx = np.arange(S, dtype=np.int32)
    orders = []
    in_maps = []
    col_start = [S] * KC
    pred_end = [0] * KC
    wqb, wkb, wvb = Wq.astype(bf), Wk.astype(bf), Wv.astype(bf)
    for b in range(B):
        vl = valid_len[b]
        vl2 = np.where(vl == 0, 1, vl).astype(np.int32)
        order = np.argsort(vl2, kind="stable")
        orders.append(order)
        vs = vl2[order]
        for kc in range(KC):
            need = vs > (kc * P)
            c0 = S if not need.any() else (int(np.argmax(need)) // P) * P
            col_start[kc] = min(col_start[kc], c0)
            full = vs >= ((kc + 1) * P)
            cv = S if not full.any() else int(np.argmax(full))
            pred_end[kc] = max(pred_end[kc], min(S, -(-cv // 32) * 32))
        in_maps.append(
            {
                "xqT": np.ascontiguousarray(query[b][order].T.astype(bf)),
                "xkT": np.ascontiguousarray(key[b].T.astype(bf)),
                "xvT": np.ascontiguousarray(value[b].T.astype(bf)),
                "wq": wqb,
                "wk": wkb,
                "wv": wvb,
                "maskT": (kidx[:, None] >= vs[None, :]).astype(np.uint8),
            }
        )
    return in_maps, orders, col_start, pred_end


def kernel(query, key, value, valid_len, Wq, Wk, Wv):
    from concourse import bass_utils

    query = np.asarray(query, dtype=np.float32)
    key = np.asarray(key, dtype=np.float32)
    value = np.asarray(value, dtype=np.float32)
    valid_len = np.asarray(valid_len, dtype=np.int32)
    Wq = np.asarray(Wq, dtype=np.float32)
    Wk = np.asarray(Wk, dtype=np.float32)
    Wv = np.asarray(Wv, dtype=np.float32)

    in_maps, orders, col_start, pred_end = _prep(
        query, key, value, valid_len, Wq, Wk, Wv
    )
    nc = _get_nc(col_start, pred_end)
    global _LAST_IN_MAPS
    _LAST_IN_MAPS = in_maps
    res = bass_utils.run_bass_kernel_spmd(nc, in_maps, core_ids=list(range(N_CORES)))

    outs = np.empty((B, S, D), dtype=np.float32)
    for b in range(B):
        raw = res.results[b]["outT"]  # [H, DH+1, S] sorted-query order
        o = raw[:, :DH, :]  # [H, DH, S]
        z = raw[:, DH, :]  # [H, S]
        o_sorted = (o / z[:, None, :]).transpose(2, 0, 1).reshape(S, D)
        inv = np.empty(S, dtype=np.int64)
        inv[orders[b]] = np.arange(S)
        outs[b] = o_sorted[inv]
        zrows = np.where(valid_len[b] == 0)[0]
        if len(zrows):
            outs[b][zrows] = value[b].mean(axis=0) @ Wv
    return outs


# revision 3
# speedup vs baseline: 1.6795x; 1.1843x over previous
"""MHA (projections + masked softmax attention) on 8 NeuronCores.

Data-parallel over batch (B=8 -> 1 batch element per core, no collectives).
bf16 matmul operands (fp32 PSUM accumulation + fp32 softmax normalization).

Per core, transposed layout:
  V  = x_v  @ Wv      [Sk, D]   (first: its output is needed by every head)
  KT = Wk^T @ x_k^T   [D, Sk]   (per 128-row chunk p, interleaved with...)
  QT = Wq^T @ x_q^T   [D, Sq]
  ...attention head-pairs at a 1-chunk lag, so the next chunk's projection
  matmuls fill the PE while the scalar engine exponentiates (PE is a strict
  FIFO: the fill must be in program order).

Attention per head h in "scores transposed" layout S^T[k, q]:
  S^T = KT_h_chunk.T @ QT_h                (k on partitions, q free)
  masked scores: copy_predicated(-480) then exp(0.125*s) => exp(-60) ~= 0
  O^T[d,q] & Z[q] in ONE accumulating matmul per 512-query half:
  lhsT = [V_h | ones] (65 cols); raw [O^T ; Z] is DMA'd out and the host
  divides / transposes / un-sorts (keeps the PE stream dense -> HAM warm).

PSUM budget (8 banks): proj acc 2x[128,512] + scores 2x[128,1024] +
attention out 2x[65,512].

Host: transposes, sort queries by valid_len (column-suffix skipping of
fully-masked key chunks + narrow predication ranges), uint8 mask, exact
fixup of valid_len==0 rows (reference -> uniform softmax -> mean(value)@Wv).
"""

import os
import sys

if "/opt/trn_rl_repo" not in sys.path:
    sys.path.insert(0, "/opt/trn_rl_repo")

import numpy as np

ABLATE = set(os.environ.get("ABLATE", "").split(","))

B, S, D, H = 8, 1024, 1024, 16
DH = D // H  # 64
P = 128
HB = 512  # half-width of the query range (1 fp32 PSUM bank)
KC = S // P  # 8 key chunks
DC = D // P  # 8 hidden chunks
N_CORES = 8
NEG = -480.0  # exp(0.125 * -480) = exp(-60) ~= 8.8e-27


def _build_nc(col_start, pred_end, reps=1):
    """col_start[kc]: first sorted-q column (mult of 128, 0..1024) needing
    key-chunk kc (1024 = chunk skipped). pred_end[kc]: end (exclusive, mult
    of 32) of the mask-predication range. Unions over cores. col_start[0]
    must be 0."""
    from contextlib import ExitStack

    import concourse.mybir as mybir
    import concourse.tile as tile
    from concourse import bacc

    fp32 = mybir.dt.float32
    bf16 = mybir.dt.bfloat16
    u8 = mybir.dt.uint8
    AF = mybir.ActivationFunctionType

    nc = bacc.Bacc(
        "TRN2",
        target_bir_lowering=False,
        debug=False,
        enable_asserts=False,
        num_devices=N_CORES,
    )

    xqT = nc.dram_tensor("xqT", (D, S), bf16, kind="ExternalInput").ap()
    xkT = nc.dram_tensor("xkT", (D, S), bf16, kind="ExternalInput").ap()
    xvT = nc.dram_tensor("xvT", (D, S), bf16, kind="ExternalInput").ap()
    wq = nc.dram_tensor("wq", (D, D), bf16, kind="ExternalInput").ap()
    wk = nc.dram_tensor("wk", (D, D), bf16, kind="ExternalInput").ap()
    wv = nc.dram_tensor("wv", (D, D), bf16, kind="ExternalInput").ap()
    maskT = nc.dram_tensor("maskT", (S, S), u8, kind="ExternalInput").ap()
    # raw per-head output: rows 0:64 = unnormalized O^T, row 64 = Z
    outT = nc.dram_tensor("outT", (H, DH + 1, S), fp32, kind="ExternalOutput").ap()

    with ExitStack() as ctx:
        tc = ctx.enter_context(tile.TileContext(nc))
        const = ctx.enter_context(tc.tile_pool(name="const", bufs=1))
        persist = ctx.enter_context(tc.tile_pool(name="persist", bufs=1))
        wpool = ctx.enter_context(tc.tile_pool(name="wpool", bufs=1))
        ppool = ctx.enter_context(tc.tile_pool(name="ppool", bufs=1, space="PSUM"))
        epool = ctx.enter_context(tc.tile_pool(name="epool", bufs=12))
        mpool = ctx.enter_context(tc.tile_pool(name="mpool", bufs=3))

        def mm(out_ap, lhsT, rhs, base, start, stop):
            # split a wide matmul into <=512-col pieces so each PE write
            # stays inside one PSUM bank. base = column offset of out_ap[0]
            # within its tile (bank alignment reference).
            w = rhs.shape[-1]
            off = 0
            while off < w:
                step = min(HB - ((base + off) % HB), w - off)
                nc.tensor.matmul(
                    out_ap[:, off : off + step],
                    lhsT,
                    rhs[:, off : off + step],
                    start=start,
                    stop=stop,
                )
                off += step

        negt = const.tile([P, S], fp32)
        nc.gpsimd.memset(negt[:], NEG)
        wrm = const.tile([P, HB], bf16)
        nc.vector.memset(wrm[:], 0.5)

        rep_cm = (
            tc.For_i(0, reps, 1, hint_engines=(mybir.EngineType.PE,))
            if reps > 1
            else None
        )
        if rep_cm is not None:
            ctx.enter_context(rep_cm)

        # HAM warmup: back-to-back dummy matmuls keep the PE busy through
        # the initial input-DMA window so the clock gate opens (K=8/8)
        # before the real matmul stream starts.
        if "nowarm" not in ABLATE:
            wps = ppool.tile([P, HB], fp32, tag="sc0", name="warm")
            for _ in range(12):
                nc.tensor.matmul(wps[:], wrm[:, :P], wrm[:], start=True, stop=True)

        qt_sb = [persist.tile([P, S], bf16, tag=f"qt{i}", name=f"qt{i}") for i in range(DC)]
        kt_sb = [persist.tile([P, S], bf16, tag=f"kt{i}", name=f"kt{i}") for i in range(DC)]
        va_sb = [persist.tile([P, H * (DH + 1)], bf16, tag=f"va{i}", name=f"va{i}") for i in range(KC)]
        mk_sb = persist.tile([P, KC * S], u8, tag="mk", name="mk")
        mk3 = mk_sb.rearrange("p (kc q) -> p kc q", q=S)

        # ---- inputs: one coarse DMA per tensor (chunk dc at cols dc*S) ----
        def load_big(dram, nm):
            t = wpool.tile([P, DC * S], bf16, tag=nm, name=nm)
            nc.sync.dma_start(
                t.rearrange("p (dc q) -> p dc q", q=S),
                dram.rearrange("(dc p) q -> p dc q", p=P),
            )
            return t

        xv2 = load_big(xvT, "xv")
        wv2 = load_big(wv, "wv")
        xk2 = load_big(xkT, "xk")
        wk2 = load_big(wk, "wk")
        xq2 = load_big(xqT, "xq")
        wq2 = load_big(wq, "wq")
        nc.sync.dma_start(
            mk3, maskT.rearrange("(kc p) q -> p kc q", p=P)
        )

        kcs = [kc for kc in range(KC) if col_start[kc] < S]

        # ---- V projection: out[k, d] per key chunk, 512-col halves ----
        if "noproj" not in ABLATE:
            nbuf = [0]

            def vproj(kc):
                va3 = va_sb[kc].rearrange("p (h d) -> p h d", d=DH + 1)
                nc.vector.memset(va3[:, :, DH], 1.0)
                for half in range(2):
                    acc = ppool.tile([P, HB], fp32, tag=f"pj{nbuf[0] % 2}", name="vacc")
                    nbuf[0] += 1
                    for dc in range(DC):
                        nc.tensor.matmul(
                            acc[:],
                            xv2[:, dc * S + kc * P : dc * S + (kc + 1) * P],
                            wv2[:, dc * D + half * HB : dc * D + (half + 1) * HB],
                            start=(dc == 0),
                            stop=(dc == DC - 1),
                        )
                    nc.scalar.copy(
                        va3[:, half * (H // 2) : (half + 1) * (H // 2), 0:DH],
                        acc[:].rearrange("p (h d) -> p h d", d=DH),
                    )

            def proj_t(w2, x2, dst, oc):
                # out[d, q] = W^T @ xT for one 128-row chunk, 512-col halves
                for half in range(2):
                    acc = ppool.tile([P, HB], fp32, tag=f"pj{nbuf[0] % 2}", name="acc")
                    nbuf[0] += 1
                    for dc in range(DC):
                        nc.tensor.matmul(
                            acc[:],
                            w2[:, dc * D + oc * P : dc * D + (oc + 1) * P],
                            x2[:, dc * S + half * HB : dc * S + (half + 1) * HB],
                            start=(dc == 0),
                            stop=(dc == DC - 1),
                        )
                    nc.vector.tensor_copy(
                        dst[oc][:, half * HB : (half + 1) * HB], acc[:]
                    )

            def attend(h):
                oc, ro = h // 2, (h % 2) * DH
                es = {}
                for i, kc in enumerate(kcs):
                    c0 = col_start[kc]
                    cv = pred_end[kc]
                    sc = ppool.tile([P, S], fp32, tag=f"sc{i % 2}", name="sc")
                    mm(sc[:, c0:], kt_sb[oc][ro : ro + DH, kc * P : (kc + 1) * P],
                       qt_sb[oc][ro : ro + DH, c0:], c0, True, True)
                    e = epool.tile([P, S], bf16, tag="e")
                    if cv > c0 and "nopred" not in ABLATE:
                        nc.vector.copy_predicated(
                            sc[:, c0:cv], mk3[:, kc, c0:cv], negt[:, : cv - c0]
                        )
                    nc.scalar.activation(e[:, c0:], sc[:, c0:], AF.Exp, scale=0.125)
                    es[kc] = e
                for half in range(2):
                    lo, hi = half * HB, (half + 1) * HB
                    ks = [kc for kc in kcs if col_start[kc] < hi]
                    at = ppool.tile(
                        [DH + 1, HB], fp32, tag=f"at{(2 * h + half) % 2}", name="at"
                    )
                    for j, kc in enumerate(ks):
                        c0 = max(col_start[kc], lo)
                        nc.tensor.matmul(
                            at[:, c0 - lo :],
                            va_sb[kc][:, h * (DH + 1) : (h + 1) * (DH + 1)],
                            es[kc][:, c0:hi],
                            start=(j == 0),
                            stop=(j == len(ks) - 1),
                        )
                    asb = mpool.tile([DH + 1, HB], fp32, tag="asb")
                    if half == 0:
                        nc.vector.tensor_copy(asb[:], at[:])
                    else:
                        nc.scalar.copy(asb[:], at[:])
                    nc.sync.dma_start(outT[h][:, lo:hi], asb[:])

            for kc in range(KC):
                vproj(kc)
            for p in range(DC):
                proj_t(wk2, xk2, kt_sb, p)
                proj_t(wq2, xq2, qt_sb, p)
                if p >= 1 and "noattn" not in ABLATE:
                    attend(2 * (p - 1))
                    attend(2 * (p - 1) + 1)
            if "noattn" not in ABLATE:
                attend(2 * (DC - 1))
                attend(2 * (DC - 1) + 1)

    nc.compile()
    return nc


_NC_CACHE = {}
_LAST_IN_MAPS = None


def _get_nc(col_start, pred_end):
    key = (tuple(col_start), tuple(pred_end))
    if key not in _NC_CACHE:
        _NC_CACHE[key] = _build_nc(list(col_start), list(pred_end))
    return _NC_CACHE[key]


def _prep(query, key, value, valid_len, Wq, Wk, Wv):
    import ml_dtypes

    bf = ml_dtypes.bfloat16
    kidx = np.arange(S, dtype=np.int32)
    orders = []
    in_maps = []
    col_start = [S] * KC
    pred_end = [0] * KC
    wqb, wkb, wvb = Wq.astype(bf), Wk.astype(bf), Wv.astype(bf)
    for b in range(B):
        vl = valid_len[b]
        vl2 = np.where(vl == 0, 1, vl).astype(np.int32)
        order = np.argsort(vl2, kind="stable")
        orders.append(order)
        vs = vl2[order]
        for kc in range(KC):
            need = vs > (kc * P)
            c0 = S if not need.any() else (int(np.argmax(need)) // P) * P
            col_start[kc] = min(col_start[kc], c0)
            full = vs >= ((kc + 1) * P)
            cv = S if not full.any() else int(np.argmax(full))
            pred_end[kc] = max(pred_end[kc], min(S, -(-cv // 32) * 32))
        in_maps.append(
            {
                "xqT": np.ascontiguousarray(query[b][order].T.astype(bf)),
                "xkT": np.ascontiguousarray(key[b].T.astype(bf)),
                "xvT": np.ascontiguousarray(value[b].T.astype(bf)),
                "wq": wqb,
                "wk": wkb,
                "wv": wvb,
                "maskT": (kidx[:, None] >= vs[None, :]).astype(np.uint8),
            }
        )
    return in_maps, orders, col_start, pred_end


def kernel(query, key, value, valid_len, Wq, Wk, Wv):
    from concourse import bass_utils

    query = np.asarray(query, dtype=np.float32)
    key = np.asarray(key, dtype=np.float32)
    value = np.asarray(value, dtype=np.float32)
    valid_len = np.asarray(valid_len, dtype=np.int32)
    Wq = np.asarray(Wq, dtype=np.float32)
    Wk = np.asarray(Wk, dtype=np.float32)
    Wv = np.asarray(Wv, dtype=np.float32)

    in_maps, orders, col_start, pred_end = _prep(
        query, key, value, valid_len, Wq, Wk, Wv
    )
    nc = _get_nc(col_start, pred_end)
    global _LAST_IN_MAPS
    _LAST_IN_MAPS = in_maps
    res = bass_utils.run_bass_kernel_spmd(nc, in_maps, core_ids=list(range(N_CORES)))

    outs = np.empty((B, S, D), dtype=np.float32)
    for b in range(B):
        raw = res.results[b]["outT"]  # [H, DH+1, S] sorted-query order
        o = raw[:, :DH, :]  # [H, DH, S]
        z = raw[:, DH, :]  # [H, S]
        o_sorted = (o / z[:, None, :]).transpose(2, 0, 1).reshape(S, D)
        inv = np.empty(S, dtype=np.int64)
        inv[orders[b]] = np.arange(S)
        outs[b] = o_sorted[inv]
        zrows = np.where(valid_len[b] == 0)[0]
        if len(zrows):
            outs[b][zrows] = value[b].mean(axis=0) @ Wv
    return outs


# revision 5
# speedup vs baseline: 1.9946x; 1.1876x over previous
"""MHA (projections + masked softmax attention) on 8 NeuronCores.

Data-parallel over batch (B=8 -> 1 batch element per core, no collectives).
bf16 matmul operands (fp32 PSUM accumulation + fp32 softmax normalization).

Per core, transposed layout:
  V  = x_v  @ Wv      [Sk, D]   (first: its output is needed by every head)
  KT = Wk^T @ x_k^T   [D, Sk]
  QT = Wq^T @ x_q^T   [D, Sq]

Attention per head-pair (2p, 2p+1) in "scores transposed" layout S^T[k, q]:
  the two heads' score matmuls use PE row-groups 0-63 / 64-127 and run
  concurrently (row tiling). exp(0.125*s) runs unmasked (scores ~N(0,1),
  no overflow), then e *= mask01 (bf16) zeroes the masked transition range.
  O^T[d,q] & Z[q] in ONE accumulating matmul per 512-query half:
  lhsT = [V_h | ones] (65 cols); raw [O^T ; Z] is DMA'd out and the host
  divides / transposes / un-sorts.

The PE is a strict FIFO, so projection matmuls for chunk p+1 are
zip-interleaved into the attention emission of pair p: the PE fills
exp-latency waits with independent projection work and the HAM clock
stays warm.

PSUM budget (8 banks): proj acc 2x[128,512] + pair scores 2x[128,1024] +
attention out 2x[65,512].

Host: transposes, sort queries by valid_len (column-suffix skipping of
fully-masked key chunks at 32-col granularity + narrow mask ranges),
bf16 0/1 mask, exact fixup of valid_len==0 rows.
"""

import os
import sys

if "/opt/trn_rl_repo" not in sys.path:
    sys.path.insert(0, "/opt/trn_rl_repo")

import numpy as np

ABLATE = set(os.environ.get("ABLATE", "").split(","))

B, S, D, H = 8, 1024, 1024, 16
DH = D // H  # 64
P = 128
HB = 512  # half-width of the query range (1 fp32 PSUM bank)
KC = S // P  # 8 key chunks
DC = D // P  # 8 hidden chunks
N_CORES = 8


def _build_nc(col_start, pred_end, reps=1):
    """col_start[kc]: first sorted-q column (mult of 32, 0..1024) needing
    key-chunk kc (1024 = chunk skipped). pred_end[kc]: end (exclusive, mult
    of 32) of the masked transition range. Unions over cores. col_start[0]
    must be 0."""
    from contextlib import ExitStack

    import concourse.mybir as mybir
    import concourse.tile as tile
    from concourse import bacc

    fp32 = mybir.dt.float32
    bf16 = mybir.dt.bfloat16
    AF = mybir.ActivationFunctionType

    nc = bacc.Bacc(
        "TRN2",
        target_bir_lowering=False,
        debug=False,
        enable_asserts=False,
        num_devices=N_CORES,
    )

    xqT = nc.dram_tensor("xqT", (D, S), bf16, kind="ExternalInput").ap()
    xkT = nc.dram_tensor("xkT", (D, S), bf16, kind="ExternalInput").ap()
    xvT = nc.dram_tensor("xvT", (D, S), bf16, kind="ExternalInput").ap()
    wq = nc.dram_tensor("wq", (D, D), bf16, kind="ExternalInput").ap()
    wk = nc.dram_tensor("wk", (D, D), bf16, kind="ExternalInput").ap()
    wv = nc.dram_tensor("wv", (D, D), bf16, kind="ExternalInput").ap()
    maskb = nc.dram_tensor("maskb", (S, S), bf16, kind="ExternalInput").ap()
    # raw per-head output: rows 0:64 = unnormalized O^T, row 64 = Z
    outT = nc.dram_tensor("outT", (H, DH + 1, S), fp32, kind="ExternalOutput").ap()

    with ExitStack() as ctx:
        tc = ctx.enter_context(tile.TileContext(nc))
        const = ctx.enter_context(tc.tile_pool(name="const", bufs=1))
        persist = ctx.enter_context(tc.tile_pool(name="persist", bufs=1))
        wpool = ctx.enter_context(tc.tile_pool(name="wpool", bufs=1))
        ppool = ctx.enter_context(tc.tile_pool(name="ppool", bufs=1, space="PSUM"))
        epool = ctx.enter_context(tc.tile_pool(name="epool", bufs=18))
        mpool = ctx.enter_context(tc.tile_pool(name="mpool", bufs=3))

        def mm(out_ap, lhsT, rhs, base, start, stop):
            # split a wide matmul into <=512-col pieces so each PE write
            # stays inside one PSUM bank. base = column offset of out_ap[0]
            # within its tile (bank alignment reference).
            w = rhs.shape[-1]
            off = 0
            while off < w:
                step = min(HB - ((base + off) % HB), w - off)
                nc.tensor.matmul(
                    out_ap[:, off : off + step],
                    lhsT,
                    rhs[:, off : off + step],
                    start=start,
                    stop=stop,
                )
                off += step

        wrm = const.tile([P, HB], bf16)
        nc.vector.memset(wrm[:], 0.5)

        rep_cm = (
            tc.For_i(0, reps, 1, hint_engines=(mybir.EngineType.PE,))
            if reps > 1
            else None
        )
        if rep_cm is not None:
            ctx.enter_context(rep_cm)

        # HAM warmup: back-to-back dummy matmuls keep the PE busy through
        # the initial input-DMA window so the clock gate opens (K=8/8)
        # before the real matmul stream starts.
        if "nowarm" not in ABLATE:
            wps = ppool.tile([P, HB], fp32, tag="sc0", name="warm")
            for _ in range(12):
                nc.tensor.matmul(wps[:], wrm[:, :P], wrm[:], start=True, stop=True)

        qt_sb = [persist.tile([P, S], bf16, tag=f"qt{i}", name=f"qt{i}") for i in range(DC)]
        kt_sb = [persist.tile([P, S], bf16, tag=f"kt{i}", name=f"kt{i}") for i in range(DC)]
        va_sb = [persist.tile([P, H * (DH + 1)], bf16, tag=f"va{i}", name=f"va{i}") for i in range(KC)]
        mk_sb = persist.tile([P, KC * S], bf16, tag="mk", name="mk")
        mk3 = mk_sb.rearrange("p (kc q) -> p kc q", q=S)

        # ---- inputs: one coarse DMA per tensor (chunk dc at cols dc*S) ----
        def load_big(dram, nm):
            t = wpool.tile([P, DC * S], bf16, tag=nm, name=nm)
            nc.sync.dma_start(
                t.rearrange("p (dc q) -> p dc q", q=S),
                dram.rearrange("(dc p) q -> p dc q", p=P),
            )
            return t

        xv2 = load_big(xvT, "xv")
        wv2 = load_big(wv, "wv")
        xk2 = load_big(xkT, "xk")
        wk2 = load_big(wk, "wk")
        xq2 = load_big(xqT, "xq")
        wq2 = load_big(wq, "wq")
        nc.sync.dma_start(mk3, maskb.rearrange("(kc p) q -> p kc q", p=P))

        kcs = [kc for kc in range(KC) if col_start[kc] < S]
        nbuf = [0]
        abuf = [0]

        # ---- V projection: out[k, d] per key chunk, 512-col halves ----
        def vproj(kc):
            va3 = va_sb[kc].rearrange("p (h d) -> p h d", d=DH + 1)
            nc.vector.memset(va3[:, :, DH], 1.0)
            for half in range(2):
                acc = ppool.tile([P, HB], fp32, tag=f"pj{nbuf[0] % 2}", name="vacc")
                nbuf[0] += 1
                for dc in range(DC):
                    nc.tensor.matmul(
                        acc[:],
                        xv2[:, dc * S + kc * P : dc * S + (kc + 1) * P],
                        wv2[:, dc * D + half * HB : dc * D + (half + 1) * HB],
                        start=(dc == 0),
                        stop=(dc == DC - 1),
                    )
                nc.scalar.copy(
                    va3[:, half * (H // 2) : (half + 1) * (H // 2), 0:DH],
                    acc[:].rearrange("p (h d) -> p h d", d=DH),
                )

        def proj_ops(p):
            """Closures emitting the K+Q projections of chunk p in ~2-MM
            steps, for zip-interleaving into the attention emission."""
            ops = []
            for w2, x2, dst in ((wk2, xk2, kt_sb), (wq2, xq2, qt_sb)):
                for half in range(2):
                    box = {}
                    for dc2 in range(0, DC, 2):
                        def step(w2=w2, x2=x2, dst=dst, half=half, dc2=dc2, box=box):
                            if dc2 == 0:
                                box["acc"] = ppool.tile(
                                    [P, HB], fp32, tag=f"pj{nbuf[0] % 2}", name="acc"
                                )
                                nbuf[0] += 1
                            for dc in (dc2, dc2 + 1):
                                nc.tensor.matmul(
                                    box["acc"][:],
                                    w2[:, dc * D + p * P : dc * D + (p + 1) * P],
                                    x2[:, dc * S + half * HB : dc * S + (half + 1) * HB],
                                    start=(dc == 0),
                                    stop=(dc == DC - 1),
                                )
                            if dc2 == DC - 2:
                                nc.vector.tensor_copy(
                                    dst[p][:, half * HB : (half + 1) * HB],
                                    box["acc"][:],
                                )
                        ops.append(step)
            return ops

        def attend_pair(p, fill):
            """Heads (2p, 2p+1): row-packed scores, exp, mask-mul, AV.
            `fill` = list of closures (projection work) drained into the
            emission to keep the PE FIFO busy during exp waits."""
            oc = p
            es = {0: {}, 1: {}}
            fi = 0
            for kc in kcs:
                c0 = col_start[kc]
                cv = pred_end[kc]
                for ro in (0, 1):  # head 2p (rows 0:64), head 2p+1 (64:128)
                    sc = ppool.tile([P, S], fp32, tag=f"sc{ro}", name="sc")
                    mm(sc[:, c0:], kt_sb[oc][ro * DH : (ro + 1) * DH, kc * P : (kc + 1) * P],
                       qt_sb[oc][ro * DH : (ro + 1) * DH, c0:], c0, True, True)
                    e = epool.tile([P, S], bf16, tag="e")
                    nc.scalar.activation(e[:, c0:], sc[:, c0:], AF.Exp, scale=0.125)
                    if cv > c0 and "nopred" not in ABLATE:
                        nc.vector.tensor_mul(
                            e[:, c0:cv], e[:, c0:cv], mk3[:, kc, c0:cv]
                        )
                    es[ro][kc] = e
                for _ in range(2):
                    if fi < len(fill):
                        fill[fi]()
                        fi += 1
            while fi < len(fill):
                fill[fi]()
                fi += 1
            for ro in (0, 1):
                h = 2 * p + ro
                for half in range(2):
                    lo, hi = half * HB, (half + 1) * HB
                    ks = [kc for kc in kcs if col_start[kc] < hi]
                    at = ppool.tile(
                        [DH + 1, HB], fp32, tag=f"at{abuf[0] % 2}", name="at"
                    )
                    abuf[0] += 1
                    for j, kc in enumerate(ks):
                        c0 = max(col_start[kc], lo)
                        nc.tensor.matmul(
                            at[:, c0 - lo :],
                            va_sb[kc][:, h * (DH + 1) : (h + 1) * (DH + 1)],
                            es[ro][kc][:, c0:hi],
                            start=(j == 0),
                            stop=(j == len(ks) - 1),
                        )
                    asb = mpool.tile([DH + 1, HB], fp32, tag="asb")
                    nc.vector.tensor_copy(asb[:], at[:])
                    nc.sync.dma_start(outT[h][:, lo:hi], asb[:])

        if "noproj" not in ABLATE:
            for kc in range(KC):
                vproj(kc)
            for op in proj_ops(0):
                op()
            if "noattn" not in ABLATE:
                for p in range(1, DC):
                    attend_pair(p - 1, proj_ops(p))
                attend_pair(DC - 1, [])

    nc.compile()
    return nc


_NC_CACHE = {}
_LAST_IN_MAPS = None


def _get_nc(col_start, pred_end):
    key = (tuple(col_start), tuple(pred_end))
    if key not in _NC_CACHE:
        _NC_CACHE[key] = _build_nc(list(col_start), list(pred_end))
    return _NC_CACHE[key]


def _prep(query, key, value, valid_len, Wq, Wk, Wv):
    import ml_dtypes

    bf = ml_dtypes.bfloat16
    kidx = np.arange(S, dtype=np.int32)
    orders = []
    in_maps = []
    col_start = [S] * KC
    pred_end = [0] * KC
    wqb, wkb, wvb = Wq.astype(bf), Wk.astype(bf), Wv.astype(bf)
    for b in range(B):
        vl = valid_len[b]
        vl2 = np.where(vl == 0, 1, vl).astype(np.int32)
        order = np.argsort(vl2, kind="stable")
        orders.append(order)
        vs = vl2[order]
        for kc in range(KC):
            need = vs > (kc * P)
            c0 = S if not need.any() else (int(np.argmax(need)) // 32) * 32
            col_start[kc] = min(col_start[kc], c0)
            full = vs >= ((kc + 1) * P)
            cv = S if not full.any() else int(np.argmax(full))
            pred_end[kc] = max(pred_end[kc], min(S, -(-cv // 32) * 32))
        in_maps.append(
            {
                "xqT": np.ascontiguousarray(query[b][order].T.astype(bf)),
                "xkT": np.ascontiguousarray(key[b].T.astype(bf)),
                "xvT": np.ascontiguousarray(value[b].T.astype(bf)),
                "wq": wqb,
                "wk": wkb,
                "wv": wvb,
                "maskb": (kidx[:, None] < vs[None, :]).astype(bf),
            }
        )
    return in_maps, orders, col_start, pred_end


def kernel(query, key, value, valid_len, Wq, Wk, Wv):
    from concourse import bass_utils

    query = np.asarray(query, dtype=np.float32)
    key = np.asarray(key, dtype=np.float32)
    value = np.asarray(value, dtype=np.float32)
    valid_len = np.asarray(valid_len, dtype=np.int32)
    Wq = np.asarray(Wq, dtype=np.float32)
    Wk = np.asarray(Wk, dtype=np.float32)
    Wv = np.asarray(Wv, dtype=np.float32)

    in_maps, orders, col_start, pred_end = _prep(
        query, key, value, valid_len, Wq, Wk, Wv
    )
    nc = _get_nc(col_start, pred_end)
    global _LAST_IN_MAPS
    _LAST_IN_MAPS = in_maps
    res = bass_utils.run_bass_kernel_spmd(nc, in_maps, core_ids=list(range(N_CORES)))

    outs = np.empty((B, S, D), dtype=np.float32)
    for b in range(B):
        raw = res.results[b]["outT"]  # [H, DH+1, S] sorted-query order
        o = raw[:, :DH, :]  # [H, DH, S]
        z = raw[:, DH, :]  # [H, S]
        o_sorted = (o / z[:, None, :]).transpose(2, 0, 1).reshape(S, D)
        inv = np.empty(S, dtype=np.int64)
        inv[orders[b]] = np.arange(S)
        outs[b] = o_sorted[inv]
        zrows = np.where(valid_len[b] == 0)[0]
        if len(zrows):
            outs[b][zrows] = value[b].mean(axis=0) @ Wv
    return outs


# revision 8
# speedup vs baseline: 2.0154x; 1.0104x over previous
"""MHA (projections + masked softmax attention) on 8 NeuronCores.

Data-parallel over batch (B=8 -> 1 batch element per core, no collectives).
bf16 matmul operands (fp32 PSUM accumulation + fp32 softmax normalization).

Per core, transposed layout:
  V  = x_v  @ Wv      [Sk, D]   (first: its output is needed by every head)
  KT = Wk^T @ x_k^T   [D, Sk]
  QT = Wq^T @ x_q^T   [D, Sq]

Attention per head-pair (2p, 2p+1) in "scores transposed" layout S^T[k, q]:
  the two heads' score matmuls use PE row-groups 0-63 / 64-127 and run
  concurrently (row tiling). exp(0.125*s) runs unmasked (scores ~N(0,1),
  no overflow), then e *= mask01 (bf16) zeroes the masked transition range.
  O^T[d,q] & Z[q] in ONE accumulating matmul per 512-query half:
  lhsT = [V_h | ones] (65 cols); raw [O^T ; Z] is DMA'd out and the host
  divides / transposes / un-sorts.

The PE is a strict FIFO, so projection matmuls for chunk p+1 are
zip-interleaved into the attention emission of pair p: the PE fills
exp-latency waits with independent projection work and the HAM clock
stays warm.

PSUM budget (8 banks): proj acc 2x[128,512] + pair scores 2x[128,1024] +
attention out 2x[65,512].

Host: transposes, sort queries by valid_len (column-suffix skipping of
fully-masked key chunks at 32-col granularity + narrow mask ranges),
bf16 0/1 mask, exact fixup of valid_len==0 rows.
"""

import os
import sys

if "/opt/trn_rl_repo" not in sys.path:
    sys.path.insert(0, "/opt/trn_rl_repo")

import numpy as np

ABLATE = set(os.environ.get("ABLATE", "").split(","))

B, S, D, H = 8, 1024, 1024, 16
DH = D // H  # 64
P = 128
HB = 512  # half-width of the query range (1 fp32 PSUM bank)
KC = S // P  # 8 key chunks
DC = D // P  # 8 hidden chunks
N_CORES = 8


def _build_nc(col_start, pred_end, reps=1):
    """col_start[kc]: first sorted-q column (mult of 32, 0..1024) needing
    key-chunk kc (1024 = chunk skipped). pred_end[kc]: end (exclusive, mult
    of 32) of the masked transition range. Unions over cores. col_start[0]
    must be 0."""
    from contextlib import ExitStack

    import concourse.mybir as mybir
    import concourse.tile as tile
    from concourse import bacc

    fp32 = mybir.dt.float32
    bf16 = mybir.dt.bfloat16
    AF = mybir.ActivationFunctionType

    nc = bacc.Bacc(
        "TRN2",
        target_bir_lowering=False,
        debug=False,
        enable_asserts=False,
        num_devices=N_CORES,
    )

    xqT = nc.dram_tensor("xqT", (D, S), bf16, kind="ExternalInput").ap()
    xkT = nc.dram_tensor("xkT", (D, S), bf16, kind="ExternalInput").ap()
    xvT = nc.dram_tensor("xvT", (D, S), bf16, kind="ExternalInput").ap()
    wq = nc.dram_tensor("wq", (D, D), bf16, kind="ExternalInput").ap()
    wk = nc.dram_tensor("wk", (D, D), bf16, kind="ExternalInput").ap()
    wv = nc.dram_tensor("wv", (D, D), bf16, kind="ExternalInput").ap()
    maskb = nc.dram_tensor("maskb", (S, S), bf16, kind="ExternalInput").ap()
    # raw per-head output: rows 0:64 = unnormalized O^T, row 64 = Z
    outT = nc.dram_tensor("outT", (H, DH + 1, S), fp32, kind="ExternalOutput").ap()

    with ExitStack() as ctx:
        tc = ctx.enter_context(tile.TileContext(nc))
        const = ctx.enter_context(tc.tile_pool(name="const", bufs=1))
        persist = ctx.enter_context(tc.tile_pool(name="persist", bufs=1))
        wpool = ctx.enter_context(tc.tile_pool(name="wpool", bufs=1))
        ppool = ctx.enter_context(tc.tile_pool(name="ppool", bufs=1, space="PSUM"))
        epool = ctx.enter_context(tc.tile_pool(name="epool", bufs=18))
        mpool = ctx.enter_context(tc.tile_pool(name="mpool", bufs=3))

        def mm(out_ap, lhsT, rhs, base, start, stop):
            # split a wide matmul into <=512-col pieces so each PE write
            # stays inside one PSUM bank. base = column offset of out_ap[0]
            # within its tile (bank alignment reference).
            w = rhs.shape[-1]
            off = 0
            while off < w:
                step = min(HB - ((base + off) % HB), w - off)
                nc.tensor.matmul(
                    out_ap[:, off : off + step],
                    lhsT,
                    rhs[:, off : off + step],
                    start=start,
                    stop=stop,
                )
                off += step

        wrm = const.tile([P, HB], bf16)
        nc.vector.memset(wrm[:], 0.5)

        rep_cm = (
            tc.For_i(0, reps, 1, hint_engines=(mybir.EngineType.PE,))
            if reps > 1
            else None
        )
        if rep_cm is not None:
            ctx.enter_context(rep_cm)

        # HAM warmup: back-to-back dummy matmuls keep the PE busy through
        # the initial input-DMA window so the clock gate opens (K=8/8)
        # before the real matmul stream starts.
        if "nowarm" not in ABLATE:
            wps = ppool.tile([P, HB], fp32, tag="sc0", name="warm")
            for _ in range(20):
                nc.tensor.matmul(wps[:], wrm[:, :P], wrm[:], start=True, stop=True)

        qt_sb = [persist.tile([P, S], bf16, tag=f"qt{i}", name=f"qt{i}") for i in range(DC)]
        kt_sb = [persist.tile([P, S], bf16, tag=f"kt{i}", name=f"kt{i}") for i in range(DC)]
        va_sb = [persist.tile([P, H * (DH + 1)], bf16, tag=f"va{i}", name=f"va{i}") for i in range(KC)]
        mk_sb = persist.tile([P, KC * S], bf16, tag="mk", name="mk")
        mk3 = mk_sb.rearrange("p (kc q) -> p kc q", q=S)

        # ---- inputs: coarse DMAs (chunk dc at cols dc*S); xv/wv split in
        # halves so the first V-projection matmuls start ~5us earlier ----
        def load_big(dram, nm, pieces=1):
            t = wpool.tile([P, DC * S], bf16, tag=nm, name=nm)
            t3 = t.rearrange("p (dc q) -> p dc q", q=S)
            d3 = dram.rearrange("(dc p) q -> p dc q", p=P)
            h = DC // pieces
            aps = []
            for i in range(pieces):
                aps.append((t3[:, i * h : (i + 1) * h, :], d3[:, i * h : (i + 1) * h, :]))
            return t, aps

        xv2, xv_aps = load_big(xvT, "xv", 2)
        wv2, wv_aps = load_big(wv, "wv", 2)
        for i in range(2):
            nc.sync.dma_start(*xv_aps[i])
            nc.sync.dma_start(*wv_aps[i])
        xk2, xk_aps = load_big(xkT, "xk")
        nc.sync.dma_start(*xk_aps[0])
        wk2, wk_aps = load_big(wk, "wk")
        nc.sync.dma_start(*wk_aps[0])
        mkb3 = maskb.rearrange("(kc p) q -> p kc q", p=P)
        nc.sync.dma_start(mk3[:, 0 : KC // 2, :], mkb3[:, 0 : KC // 2, :])
        xq2, xq_aps = load_big(xqT, "xq")
        nc.sync.dma_start(*xq_aps[0])
        wq2, wq_aps = load_big(wq, "wq")
        nc.sync.dma_start(*wq_aps[0])
        nc.sync.dma_start(mk3[:, KC // 2 : KC, :], mkb3[:, KC // 2 : KC, :])

        kcs = [kc for kc in range(KC) if col_start[kc] < S]
        nbuf = [0]
        abuf = [0]

        # ---- V projection: out[k, d] per key chunk, 512-col halves ----
        def vproj(kc):
            va3 = va_sb[kc].rearrange("p (h d) -> p h d", d=DH + 1)
            nc.vector.memset(va3[:, :, DH], 1.0)
            for half in range(2):
                acc = ppool.tile([P, HB], fp32, tag=f"pj{nbuf[0] % 2}", name="vacc")
                nbuf[0] += 1
                for dc in range(DC):
                    nc.tensor.matmul(
                        acc[:],
                        xv2[:, dc * S + kc * P : dc * S + (kc + 1) * P],
                        wv2[:, dc * D + half * HB : dc * D + (half + 1) * HB],
                        start=(dc == 0),
                        stop=(dc == DC - 1),
                    )
                nc.scalar.copy(
                    va3[:, half * (H // 2) : (half + 1) * (H // 2), 0:DH],
                    acc[:].rearrange("p (h d) -> p h d", d=DH),
                )

        def proj_ops(p):
            """Closures emitting the K+Q projections of chunk p in ~2-MM
            steps, for zip-interleaving into the attention emission."""
            ops = []
            for w2, x2, dst in ((wk2, xk2, kt_sb), (wq2, xq2, qt_sb)):
                for half in range(2):
                    box = {}
                    for dc2 in range(0, DC, 2):
                        def step(w2=w2, x2=x2, dst=dst, half=half, dc2=dc2, box=box):
                            if dc2 == 0:
                                box["acc"] = ppool.tile(
                                    [P, HB], fp32, tag=f"pj{nbuf[0] % 2}", name="acc"
                                )
                                nbuf[0] += 1
                            for dc in (dc2, dc2 + 1):
                                nc.tensor.matmul(
                                    box["acc"][:],
                                    w2[:, dc * D + p * P : dc * D + (p + 1) * P],
                                    x2[:, dc * S + half * HB : dc * S + (half + 1) * HB],
                                    start=(dc == 0),
                                    stop=(dc == DC - 1),
                                )
                            if dc2 == DC - 2:
                                nc.vector.tensor_copy(
                                    dst[p][:, half * HB : (half + 1) * HB],
                                    box["acc"][:],
                                )
                        ops.append(step)
            return ops

        def attend_pair(p, fill):
            """Heads (2p, 2p+1): row-packed scores, exp, mask-mul, AV.
            `fill` = list of closures (projection work) drained into the
            emission to keep the PE FIFO busy during exp waits."""
            oc = p
            es = {0: {}, 1: {}}
            fi = 0
            for kc in kcs:
                c0 = col_start[kc]
                cv = pred_end[kc]
                for ro in (0, 1):  # head 2p (rows 0:64), head 2p+1 (64:128)
                    sc = ppool.tile([P, S], fp32, tag=f"sc{ro}", name="sc")
                    mm(sc[:, c0:], kt_sb[oc][ro * DH : (ro + 1) * DH, kc * P : (kc + 1) * P],
                       qt_sb[oc][ro * DH : (ro + 1) * DH, c0:], c0, True, True)
                    e = epool.tile([P, S], bf16, tag="e")
                    nc.scalar.activation(e[:, c0:], sc[:, c0:], AF.Exp, scale=0.125)
                    if cv > c0 and "nopred" not in ABLATE:
                        nc.vector.tensor_mul(
                            e[:, c0:cv], e[:, c0:cv], mk3[:, kc, c0:cv]
                        )
                    es[ro][kc] = e
                for _ in range(2):
                    if fi < len(fill):
                        fill[fi]()
                        fi += 1
            while fi < len(fill):
                fill[fi]()
                fi += 1
            for ro in (0, 1):
                h = 2 * p + ro
                for half in range(2):
                    lo, hi = half * HB, (half + 1) * HB
                    ks = [kc for kc in kcs if col_start[kc] < hi]
                    at = ppool.tile(
                        [DH + 1, HB], fp32, tag=f"at{abuf[0] % 2}", name="at"
                    )
                    abuf[0] += 1
                    for j, kc in enumerate(ks):
                        c0 = max(col_start[kc], lo)
                        nc.tensor.matmul(
                            at[:, c0 - lo :],
                            va_sb[kc][:, h * (DH + 1) : (h + 1) * (DH + 1)],
                            es[ro][kc][:, c0:hi],
                            start=(j == 0),
                            stop=(j == len(ks) - 1),
                        )
                    asb = mpool.tile([DH + 1, HB], fp32, tag="asb")
                    nc.vector.tensor_copy(asb[:], at[:])
                    nc.sync.dma_start(outT[h][:, lo:hi], asb[:])

        if "noproj" not in ABLATE:
            for kc in range(KC):
                vproj(kc)
            for op in proj_ops(0):
                op()
            if "noattn" not in ABLATE:
                for p in range(1, DC):
                    attend_pair(p - 1, proj_ops(p))
                attend_pair(DC - 1, [])

    nc.compile()
    return nc


_NC_CACHE = {}
_LAST_IN_MAPS = None


def _get_nc(col_start, pred_end):
    key = (tuple(col_start), tuple(pred_end))
    if key not in _NC_CACHE:
        _NC_CACHE[key] = _build_nc(list(col_start), list(pred_end))
    return _NC_CACHE[key]


def _prep(query, key, value, valid_len, Wq, Wk, Wv):
    import ml_dtypes

    bf = ml_dtypes.bfloat16
    kidx = np.arange(S, dtype=np.int32)
    orders = []
    in_maps = []
    col_start = [S] * KC
    pred_end = [0] * KC
    wqb, wkb, wvb = Wq.astype(bf), Wk.astype(bf), Wv.astype(bf)
    for b in range(B):
        vl = valid_len[b]
        vl2 = np.where(vl == 0, 1, vl).astype(np.int32)
        order = np.argsort(vl2, kind="stable")
        orders.append(order)
        vs = vl2[order]
        for kc in range(KC):
            need = vs > (kc * P)
            c0 = S if not need.any() else (int(np.argmax(need)) // 32) * 32
            col_start[kc] = min(col_start[kc], c0)
            full = vs >= ((kc + 1) * P)
            cv = S if not full.any() else int(np.argmax(full))
            pred_end[kc] = max(pred_end[kc], min(S, -(-cv // 32) * 32))
        in_maps.append(
            {
                "xqT": np.ascontiguousarray(query[b][order].T.astype(bf)),
                "xkT": np.ascontiguousarray(key[b].T.astype(bf)),
                "xvT": np.ascontiguousarray(value[b].T.astype(bf)),
                "wq": wqb,
                "wk": wkb,
                "wv": wvb,
                "maskb": (kidx[:, None] < vs[None, :]).astype(bf),
            }
        )
    return in_maps, orders, col_start, pred_end


def kernel(query, key, value, valid_len, Wq, Wk, Wv):
    from concourse import bass_utils

    query = np.asarray(query, dtype=np.float32)
    key = np.asarray(key, dtype=np.float32)
    value = np.asarray(value, dtype=np.float32)
    valid_len = np.asarray(valid_len, dtype=np.int32)
    Wq = np.asarray(Wq, dtype=np.float32)
    Wk = np.asarray(Wk, dtype=np.float32)
    Wv = np.asarray(Wv, dtype=np.float32)

    in_maps, orders, col_start, pred_end = _prep(
        query, key, value, valid_len, Wq, Wk, Wv
    )
    nc = _get_nc(col_start, pred_end)
    global _LAST_IN_MAPS
    _LAST_IN_MAPS = in_maps
    res = bass_utils.run_bass_kernel_spmd(nc, in_maps, core_ids=list(range(N_CORES)))

    outs = np.empty((B, S, D), dtype=np.float32)
    for b in range(B):
        raw = res.results[b]["outT"]  # [H, DH+1, S] sorted-query order
        o = raw[:, :DH, :]  # [H, DH, S]
        z = raw[:, DH, :]  # [H, S]
        o_sorted = (o / z[:, None, :]).transpose(2, 0, 1).reshape(S, D)
        inv = np.empty(S, dtype=np.int64)
        inv[orders[b]] = np.arange(S)
        outs[b] = o_sorted[inv]
        zrows = np.where(valid_len[b] == 0)[0]
        if len(zrows):
            outs[b][zrows] = value[b].mean(axis=0) @ Wv
    return outs
